# revision 16
# baseline (speedup 1.0000x reference)
"""Trainium2 Bass kernel for ChronoRotationTransformation.

Computes, per batch row b (B=8192, D=2048):
    u   = (head_r + i*head_i) * (rel_r + i*rel_i)          # complex product
    ab  = sum_d u_r*tail_r - u_i*tail_i                    # == sum rot_r*t_r + rot_i*t_i
    aa  = sum_d u_r^2 + u_i^2                              # == |rot|^2
    bb  = sum_d tail_r^2 + tail_i^2
    out = ab / sqrt(aa*bb)

(The reference's rot = conj(head*rel); rot_r = u_r, rot_i = -u_i, so
ab = rot_r*t_r + rot_i*t_i = u_r*t_r - u_i*t_i and |rot|^2 = |u|^2.)

Sharding: pure data-parallel across 8 NeuronCores, 1024 rows each.
Per core: 8 row-tiles of [128, 2048]. Inputs are streamed as fp16
(host-side cast; quantization costs 5.0e-4 scale-relative absmax vs
the fp32 reference — 40x inside the 2e-2 gate) which halves HBM
traffic to 24 MiB/core (~70-76us DMA roofline). All accumulation is
fp32. DVE does the 4 cross products, the two add/subs forming u, and
two fused multiply+reduce STTs (ab); ACT does 4 square+accumulate
reductions (aa, bb). At fp16 DVE is the binding engine (~11.1us/tile:
TT ops get the 2x 2-byte mode, the two STTs don't); measured
~90us/rep vs 146.6us for the fp32-streamed version.
"""

import numpy as np

B, D = 8192, 2048
NCORES = 8
BC = B // NCORES            # rows per core
P = 128                     # SBUF partitions
NT = BC // P                # row-tiles per core

IN_NAMES = [
    "head_real", "head_imag",
    "rel_real", "rel_imag",
    "tail_real", "tail_imag",
]

# Streamed-input dtype. The score is a normalized correlation; fp16
# input quantization costs ~3.4e-4 scale-relative absmax (measured vs
# the fp32 reference) — far inside the 2e-2 gate — and halves HBM
# traffic, which is the binding roofline for this kernel.
DEFAULT_CFG = "f16"

_CACHE = {}


def _emit(tc, ins, out_ap, mybir, repeats=1, cfg=DEFAULT_CFG):
    import concourse.bass as bass  # noqa: F401

    nc = tc.nc
    f32 = mybir.dt.float32
    sdt = mybir.dt.float16 if cfg.startswith("f16") else f32
    ibufs = 4 if cfg.startswith("f16") else 2
    Alu = mybir.AluOpType
    Act = mybir.ActivationFunctionType

    # DRAM views: [NT, P, D] row-tiles; out as [P, NT] (row = t*128 + p).
    dv = {n: ins[n].rearrange("(t p) d -> t p d", p=P) for n in IN_NAMES}
    out_d = out_ap.rearrange("(t p) -> p t", p=P)

    with (
        tc.tile_pool(name="inp", bufs=2) as inp,
        tc.tile_pool(name="prod", bufs=1) as prod,
        tc.tile_pool(name="upool", bufs=2) as upool,
        tc.tile_pool(name="scr", bufs=1) as scr,
        tc.tile_pool(name="stats", bufs=1) as stats,
    ):
        for _rep in range(repeats):
          # Per-rep stats tiles, double-buffered: rep k+1's accum writes
          # (ACT/Pool engines) must not wait on rep k's final-combine
          # reads (DVE) — with bufs=2 they land in the other buffer.
          ab1_s = stats.tile([P, NT], f32, tag="ab1_s", bufs=2)
          ab2_s = stats.tile([P, NT], f32, tag="ab2_s", bufs=2)
          aa1_s = stats.tile([P, NT], f32, tag="aa1_s", bufs=2)
          aa2_s = stats.tile([P, NT], f32, tag="aa2_s", bufs=2)
          bb1_s = stats.tile([P, NT], f32, tag="bb1_s", bufs=2)
          bb2_s = stats.tile([P, NT], f32, tag="bb2_s", bufs=2)
          for t in range(NT):
            tiles = {}
            for n in IN_NAMES:
                # tail tiles are the last-released each tile (read by the
                # STT dots at the end) — give them one extra buffer so
                # their next DMA isn't gated on the ring.
                nb = ibufs + 1 if n.startswith("tail") else ibufs
                tl = inp.tile([P, D], sdt, tag=n, bufs=nb)
                nc.sync.dma_start(out=tl[:], in_=dv[n][t])
                tiles[n] = tl
            hr, hi = tiles["head_real"], tiles["head_imag"]
            rr, ri = tiles["rel_real"], tiles["rel_imag"]
            tr, ti = tiles["tail_real"], tiles["tail_imag"]

            # All products on DVE. Tried offloading m3/m4 to GPSIMD/Pool
            # (cost model says 0.42 eff ≈ 4.1us/op, Pool otherwise idle):
            # measured 136us/rep vs 88.7 all-DVE — GPSIMD fp16 TT is far
            # slower than modeled and the Pool→DVE handoff serializes the
            # pipeline. STT on Pool doesn't even compile ("Instruction
            # engine check failed"). DVE-bound at ~11.1us/tile is the
            # practical floor for this op set (STT gets no 2-byte mode).
            m3 = prod.tile([P, D], sdt, tag="m3", bufs=2)
            nc.vector.tensor_mul(m3[:], hi[:], rr[:])
            m4 = prod.tile([P, D], sdt, tag="m4", bufs=2)
            nc.vector.tensor_mul(m4[:], hr[:], ri[:])
            m1 = prod.tile([P, D], sdt, tag="m1")
            nc.vector.tensor_mul(m1[:], hr[:], rr[:])
            m2 = prod.tile([P, D], sdt, tag="m2")
            nc.vector.tensor_mul(m2[:], hi[:], ri[:])
            ub = 3
            ur = upool.tile([P, D], sdt, tag="ur", bufs=ub)
            nc.vector.tensor_sub(ur[:], m1[:], m2[:])
            ui = upool.tile([P, D], sdt, tag="ui", bufs=ub)
            nc.vector.tensor_add(ui[:], m3[:], m4[:])

            # ab = sum(ur*tr) - sum(ui*ti): fused multiply+reduce via
            # scalar_tensor_tensor (out = (in0 op0 scalar) op1 in1,
            # accum_out = sum(out)). tensor_tensor_reduce (native TTR
            # opcode) crashes this terminal's NRT — do not use it.
            # STT gets no 2-byte DVE speedup (no perf modes), so at fp16
            # DVE is the bottleneck engine (8x 10240cy = 85us vs ~75us
            # DMA): keep one dot on DVE (scratch out aliases the dead m1
            # slot — WAR stays on-engine) and push the other onto the
            # otherwise-idle GPSIMD/Pool engine (~2.9us/tile at 0.6
            # efficiency, huge slack).
            so1 = prod.tile([P, D], sdt, tag="m1")
            nc.vector.scalar_tensor_tensor(
                out=so1[:], in0=ur[:], scalar=1.0, in1=tr[:],
                op0=Alu.mult, op1=Alu.mult, accum_out=ab1_s[:, t:t + 1],
            )
            # ab2 = sum(ui*ti): the STT opcode runs 1 elem/cycle, so at
            # fp16 it pays to split this dot — multiply on DVE as a
            # plain TT (2x 2-byte mode, 1024cy instead of 2048) and
            # reduce on ACT via Copy+accum (Copy shares the sqrt act
            # table — no table reload). Rebalances DVE 92->84us/core
            # against ACT 62->78us/core.
            p2 = prod.tile([P, D], sdt, tag="m2")
            nc.vector.tensor_mul(p2[:], ui[:], ti[:])
            ao2 = scr.tile([P, D], sdt, tag="ao")
            nc.scalar.activation(
                out=ao2[:], in_=p2[:], func=Act.Copy,
                accum_out=ab2_s[:, t:t + 1],
            )

            # aa, bb: square+accumulate on ACT. bb first — tr/ti are
            # already resident before DVE finishes the products, so ACT
            # starts early and tr/ti stay hot for the STT dots.
            for src, dst in (
                (tr, bb1_s), (ti, bb2_s), (ur, aa1_s), (ui, aa2_s),
            ):
                ao = scr.tile([P, D], sdt, tag="ao")
                nc.scalar.activation(
                    out=ao[:], in_=src[:], func=Act.Square,
                    accum_out=dst[:, t:t + 1],
                )

          # Final combine on [P, NT] (tiny); inside the rep loop so a
          # timed rep is the complete kernel.
          def ftile(name):
              tl = stats.tile([P, NT], f32, tag=name, bufs=2)
              return tl

          # ab2_s now holds +sum(ui*ti) (no -1 scalar in the ACT path).
          ab = ftile("ab"); nc.vector.tensor_sub(ab[:], ab1_s[:], ab2_s[:])
          aa = ftile("aa"); nc.vector.tensor_add(aa[:], aa1_s[:], aa2_s[:])
          bb = ftile("bb"); nc.vector.tensor_add(bb[:], bb1_s[:], bb2_s[:])
          pp = ftile("pp"); nc.vector.tensor_mul(pp[:], aa[:], bb[:])
          # sqrt on ACT is low precision (up to ~65536 ULP ≈ 8e-3 rel);
          # one Newton iteration  r <- 0.5*(r + p/r)  with the bit-exact
          # DVE reciprocal squares that to ~1e-5 — far below the fp16
          # input-quantization error floor (~5e-4).
          r = ftile("r0"); nc.scalar.activation(out=r[:], in_=pp[:], func=Act.Sqrt)
          q = ftile("q0"); nc.vector.reciprocal(q[:], r[:])
          pq = ftile("pq0"); nc.vector.tensor_mul(pq[:], pp[:], q[:])
          s = ftile("s0"); nc.vector.tensor_add(s[:], r[:], pq[:])
          r = ftile("r1"); nc.vector.tensor_scalar_mul(r[:], s[:], 0.5)
          inv = ftile("inv"); nc.vector.reciprocal(inv[:], r[:])
          score = ftile("score"); nc.vector.tensor_mul(score[:], ab[:], inv[:])
          nc.sync.dma_start(out=out_d, in_=score[:])


def _build(repeats=1, cfg=DEFAULT_CFG):
    key = ("nc", repeats, cfg)
    if key in _CACHE:
        return _CACHE[key]
    import concourse.tile as tile
    from concourse import bacc, mybir

    # NOTE: num_devices is deliberately NOT set — it enables collective
    # global-comm setup that breaks plain SPMD input binding under the
    # axon/PJRT path (outputs come back as garbage).
    nc = bacc.Bacc(
        "TRN2",
        target_bir_lowering=False,
        debug=False,
    )
    sdt = mybir.dt.float16 if cfg.startswith("f16") else mybir.dt.float32
    ins = {
        n: nc.dram_tensor(n, [BC, D], sdt, kind="ExternalInput").ap()
        for n in IN_NAMES
    }
    out = nc.dram_tensor("out", [BC], mybir.dt.float32, kind="ExternalOutput").ap()
    with tile.TileContext(nc) as tc:
        _emit(tc, ins, out, mybir, repeats=repeats, cfg=cfg)
    nc.compile()
    _CACHE[key] = nc
    return nc


def _shard(inputs, cfg=DEFAULT_CFG):
    np_dt = np.float16 if cfg.startswith("f16") else np.float32
    in_maps = []
    for c in range(NCORES):
        sl = slice(c * BC, (c + 1) * BC)
        in_maps.append(
            {n: np.ascontiguousarray(inputs[n][sl], dtype=np_dt)
             for n in IN_NAMES}
        )
    return in_maps


def run(inputs, trace=False, **kwargs):
    """Run on 8 cores; returns (full_output, BassKernelResults)."""
    from concourse.bass_utils import run_bass_kernel_spmd

    nc = _build()
    core_ids = list(range(NCORES))
    in_maps = _shard(inputs)
    # The terminal occasionally reports the accelerator unrecoverable
    # (e.g. poisoned by an earlier crashed run); a fresh attempt after a
    # short wait triggers recovery.
    last_exc = None
    for attempt in range(4):
        try:
            res = run_bass_kernel_spmd(nc, in_maps, core_ids, trace=trace, **kwargs)
            break
        except Exception as e:  # noqa: BLE001
            last_exc = e
            if attempt == 3:
                raise
            import time as _time
            _time.sleep(15 * (attempt + 1))
    out = np.concatenate([res.results[c]["out"] for c in range(NCORES)])
    return out.astype(np.float32), res


def kernel(**inputs):
    out, _ = run(inputs)
    return out



# revision 18
# speedup vs baseline: 1.1635x; 1.1635x over previous
"""Trainium2 Bass kernel for ChronoRotationTransformation.

Computes, per batch row b (B=8192, D=2048):
    u   = (head_r + i*head_i) * (rel_r + i*rel_i)          # complex product
    ab  = sum_d u_r*tail_r - u_i*tail_i                    # == sum rot_r*t_r + rot_i*t_i
    aa  = sum_d u_r^2 + u_i^2                              # == |rot|^2
    bb  = sum_d tail_r^2 + tail_i^2
    out = ab / sqrt(aa*bb)

(The reference's rot = conj(head*rel); rot_r = u_r, rot_i = -u_i, so
ab = rot_r*t_r + rot_i*t_i = u_r*t_r - u_i*t_i and |rot|^2 = |u|^2.)

Sharding: pure data-parallel across 8 NeuronCores, 1024 rows each.
Per core: 8 row-tiles of [128, 2048]. Inputs are streamed as fp16
(host-side cast; quantization costs 5.0e-4 scale-relative absmax vs
the fp32 reference — 40x inside the 2e-2 gate) which halves HBM
traffic to 24 MiB/core (~70-76us DMA roofline). All accumulation is
fp32. DVE does the 4 cross products, the two add/subs forming u, and
two fused multiply+reduce STTs (ab); ACT does 4 square+accumulate
reductions (aa, bb). At fp16 DVE is the binding engine (~11.1us/tile:
TT ops get the 2x 2-byte mode, the two STTs don't); measured
~90us/rep vs 146.6us for the fp32-streamed version.
"""

import numpy as np

B, D = 8192, 2048
NCORES = 8
BC = B // NCORES            # rows per core
P = 128                     # SBUF partitions
NT = BC // P                # row-tiles per core

IN_NAMES = [
    "head_real", "head_imag",
    "rel_real", "rel_imag",
    "tail_real", "tail_imag",
]

# Streamed-input dtype. The score is a normalized correlation; fp16
# input quantization costs ~3.4e-4 scale-relative absmax (measured vs
# the fp32 reference) — far inside the 2e-2 gate — and halves HBM
# traffic, which is the binding roofline for this kernel.
DEFAULT_CFG = "f16"

_CACHE = {}


def _emit(tc, ins, out_ap, mybir, repeats=1, cfg=DEFAULT_CFG):
    import concourse.bass as bass  # noqa: F401

    nc = tc.nc
    f32 = mybir.dt.float32
    sdt = mybir.dt.float16 if cfg.startswith("f16") else f32
    ibufs = 4 if cfg.startswith("f16") else 2
    Alu = mybir.AluOpType
    Act = mybir.ActivationFunctionType

    # DRAM views: [NT, P, D] row-tiles; out as [P, NT] (row = t*128 + p).
    dv = {n: ins[n].rearrange("(t p) d -> t p d", p=P) for n in IN_NAMES}
    out_d = out_ap.rearrange("(t p) -> p t", p=P)

    with (
        tc.tile_pool(name="inp", bufs=2) as inp,
        tc.tile_pool(name="prod", bufs=1) as prod,
        tc.tile_pool(name="upool", bufs=2) as upool,
        tc.tile_pool(name="scr", bufs=1) as scr,
        tc.tile_pool(name="stats", bufs=1) as stats,
    ):
        for _rep in range(repeats):
          # Per-rep stats tiles, double-buffered: rep k+1's accum writes
          # (ACT/Pool engines) must not wait on rep k's final-combine
          # reads (DVE) — with bufs=2 they land in the other buffer.
          ab1_s = stats.tile([P, NT], f32, tag="ab1_s", bufs=2)
          ab2_s = stats.tile([P, NT], f32, tag="ab2_s", bufs=2)
          aa1_s = stats.tile([P, NT], f32, tag="aa1_s", bufs=2)
          aa2_s = stats.tile([P, NT], f32, tag="aa2_s", bufs=2)
          bb1_s = stats.tile([P, NT], f32, tag="bb1_s", bufs=2)
          bb2_s = stats.tile([P, NT], f32, tag="bb2_s", bufs=2)
          for t in range(NT):
            tiles = {}
            for n in IN_NAMES:
                # tail tiles are the last-released each tile (read by the
                # STT dots at the end) — give them one extra buffer so
                # their next DMA isn't gated on the ring.
                nb = ibufs + 1 if n.startswith("tail") else ibufs
                tl = inp.tile([P, D], sdt, tag=n, bufs=nb)
                nc.sync.dma_start(out=tl[:], in_=dv[n][t])
                tiles[n] = tl
            hr, hi = tiles["head_real"], tiles["head_imag"]
            rr, ri = tiles["rel_real"], tiles["rel_imag"]
            tr, ti = tiles["tail_real"], tiles["tail_imag"]

            # All products on DVE. Tried offloading m3/m4 to GPSIMD/Pool
            # (cost model says 0.42 eff ≈ 4.1us/op, Pool otherwise idle):
            # measured 136us/rep vs 88.7 all-DVE — GPSIMD fp16 TT is far
            # slower than modeled and the Pool→DVE handoff serializes the
            # pipeline. STT on Pool doesn't even compile ("Instruction
            # engine check failed"). DVE-bound at ~11.1us/tile is the
            # practical floor for this op set (STT gets no 2-byte mode).
            m3 = prod.tile([P, D], sdt, tag="m3", bufs=2)
            nc.vector.tensor_mul(m3[:], hi[:], rr[:])
            m4 = prod.tile([P, D], sdt, tag="m4", bufs=2)
            nc.vector.tensor_mul(m4[:], hr[:], ri[:])
            m1 = prod.tile([P, D], sdt, tag="m1")
            nc.vector.tensor_mul(m1[:], hr[:], rr[:])
            m2 = prod.tile([P, D], sdt, tag="m2")
            nc.vector.tensor_mul(m2[:], hi[:], ri[:])
            ub = 3
            ur = upool.tile([P, D], sdt, tag="ur", bufs=ub)
            nc.vector.tensor_sub(ur[:], m1[:], m2[:])
            ui = upool.tile([P, D], sdt, tag="ui", bufs=ub)
            nc.vector.tensor_add(ui[:], m3[:], m4[:])

            # ab = sum(ur*tr) - sum(ui*ti): fused multiply+reduce via
            # scalar_tensor_tensor (out = (in0 op0 scalar) op1 in1,
            # accum_out = sum(out)). tensor_tensor_reduce (native TTR
            # opcode) crashes this terminal's NRT — do not use it.
            # STT gets no 2-byte DVE speedup (no perf modes), so these
            # two are the DVE bottleneck's 4096 of 10240 cy/tile. Both
            # alternatives measured WORSE: STT on Pool fails the backend
            # engine check outright, and splitting a dot into TT-mul +
            # ACT Copy+accum measured 102us/rep (ACT accumulator reads
            # + the late DVE->ACT handoff cost more than the model
            # says). Scratch outs alias the dead m1/m2 slots — WAR
            # stays on-engine, zero extra SBUF.
            so1 = prod.tile([P, D], sdt, tag="m1")
            nc.vector.scalar_tensor_tensor(
                out=so1[:], in0=ur[:], scalar=1.0, in1=tr[:],
                op0=Alu.mult, op1=Alu.mult, accum_out=ab1_s[:, t:t + 1],
            )
            so2 = prod.tile([P, D], sdt, tag="m2")
            nc.vector.scalar_tensor_tensor(
                out=so2[:], in0=ui[:], scalar=-1.0, in1=ti[:],
                op0=Alu.mult, op1=Alu.mult, accum_out=ab2_s[:, t:t + 1],
            )

            # aa, bb: square+accumulate on ACT. bb first — tr/ti are
            # already resident before DVE finishes the products, so ACT
            # starts early and tr/ti stay hot for the STT dots.
            for src, dst in (
                (tr, bb1_s), (ti, bb2_s), (ur, aa1_s), (ui, aa2_s),
            ):
                ao = scr.tile([P, D], sdt, tag="ao")
                nc.scalar.activation(
                    out=ao[:], in_=src[:], func=Act.Square,
                    accum_out=dst[:, t:t + 1],
                )

          # Final combine on [P, NT] (tiny); inside the rep loop so a
          # timed rep is the complete kernel.
          def ftile(name):
              tl = stats.tile([P, NT], f32, tag=name, bufs=2)
              return tl

          ab = ftile("ab"); nc.vector.tensor_add(ab[:], ab1_s[:], ab2_s[:])
          aa = ftile("aa"); nc.vector.tensor_add(aa[:], aa1_s[:], aa2_s[:])
          bb = ftile("bb"); nc.vector.tensor_add(bb[:], bb1_s[:], bb2_s[:])
          pp = ftile("pp"); nc.vector.tensor_mul(pp[:], aa[:], bb[:])
          # sqrt on ACT is low precision (up to ~65536 ULP ≈ 8e-3 rel);
          # one Newton iteration  r <- 0.5*(r + p/r)  with the bit-exact
          # DVE reciprocal squares that to ~1e-5 — far below the fp16
          # input-quantization error floor (~5e-4).
          r = ftile("r0"); nc.scalar.activation(out=r[:], in_=pp[:], func=Act.Sqrt)
          q = ftile("q0"); nc.vector.reciprocal(q[:], r[:])
          pq = ftile("pq0"); nc.vector.tensor_mul(pq[:], pp[:], q[:])
          s = ftile("s0"); nc.vector.tensor_add(s[:], r[:], pq[:])
          r = ftile("r1"); nc.vector.tensor_scalar_mul(r[:], s[:], 0.5)
          inv = ftile("inv"); nc.vector.reciprocal(inv[:], r[:])
          score = ftile("score"); nc.vector.tensor_mul(score[:], ab[:], inv[:])
          nc.sync.dma_start(out=out_d, in_=score[:])


def _build(repeats=1, cfg=DEFAULT_CFG):
    key = ("nc", repeats, cfg)
    if key in _CACHE:
        return _CACHE[key]
    import concourse.tile as tile
    from concourse import bacc, mybir

    # NOTE: num_devices is deliberately NOT set — it enables collective
    # global-comm setup that breaks plain SPMD input binding under the
    # axon/PJRT path (outputs come back as garbage).
    nc = bacc.Bacc(
        "TRN2",
        target_bir_lowering=False,
        debug=False,
    )
    sdt = mybir.dt.float16 if cfg.startswith("f16") else mybir.dt.float32
    ins = {
        n: nc.dram_tensor(n, [BC, D], sdt, kind="ExternalInput").ap()
        for n in IN_NAMES
    }
    out = nc.dram_tensor("out", [BC], mybir.dt.float32, kind="ExternalOutput").ap()
    with tile.TileContext(nc) as tc:
        _emit(tc, ins, out, mybir, repeats=repeats, cfg=cfg)
    nc.compile()
    _CACHE[key] = nc
    return nc


def _shard(inputs, cfg=DEFAULT_CFG):
    np_dt = np.float16 if cfg.startswith("f16") else np.float32
    in_maps = []
    for c in range(NCORES):
        sl = slice(c * BC, (c + 1) * BC)
        in_maps.append(
            {n: np.ascontiguousarray(inputs[n][sl], dtype=np_dt)
             for n in IN_NAMES}
        )
    return in_maps


def run(inputs, trace=False, **kwargs):
    """Run on 8 cores; returns (full_output, BassKernelResults)."""
    from concourse.bass_utils import run_bass_kernel_spmd

    nc = _build()
    core_ids = list(range(NCORES))
    in_maps = _shard(inputs)
    # The terminal occasionally reports the accelerator unrecoverable
    # (e.g. poisoned by an earlier crashed run); a fresh attempt after a
    # short wait triggers recovery.
    last_exc = None
    for attempt in range(4):
        try:
            res = run_bass_kernel_spmd(nc, in_maps, core_ids, trace=trace, **kwargs)
            break
        except Exception as e:  # noqa: BLE001
            last_exc = e
            if attempt == 3:
                raise
            import time as _time
            _time.sleep(15 * (attempt + 1))
    out = np.concatenate([res.results[c]["out"] for c in range(NCORES)])
    return out.astype(np.float32), res


def kernel(**inputs):
    out, _ = run(inputs)
    return out

